# revision 1
# baseline (speedup 1.0000x reference)
"""Trainium2 Bass kernel for nn_CausalSelfAttention_5411658793445.

Sharding: queries (token dim) split 8 ways; K/V projection also token-split,
with the current block's roped K / V exchanged via AllGather so every core
attends over the full kept KV window (prior cache slice + current block).

Per-core device program (identical SPMD program, per-core data):
  1. fp32r projections of the core's 330-token slice: kT [d,t], vT (direct
     [t,d]), qT [d,t]
  2. RMSNorm (partition-dim sum-of-squares via ones-matmul) + RoPE (pair
     components deinterleaved into partition halves by a host-side weight-row
     permutation; the cross-half combine uses a PE half-swap matmul)
  3. AllGather of current roped K [d,t] and V [t,d] (bf16) across 8 cores
  4. Attention, scores-transposed layout: for each head, l-tiles of 128 kept
     positions: PE scores [l,330] -> ACT exp (bf16) -> PE (escT as stationary)
     x [V | ones-column] accumulating numerator AND denominator in one psum
     [110,129] x 3 s-subtiles.  Zero-padded KV rows contribute exactly 1.0 to
     the denominator each and 0 to the numerator -> one constant correction.
  5. divide, PE-transpose to [d,t], fp32r output projection, + bo -> out rows.
"""

import math
from contextlib import ExitStack

import numpy as np
import ml_dtypes

NC = 8
DIM, NH, HD = 1536, 12, 128
HALF = 64
H, W = 22, 40
FRAME = H * W            # 880
S_TOTAL = 2640
SC = S_TOTAL // NC       # 330
ST = 110                 # s-subtile (330 = 3*110)
SCP = 384                # padded per-core token count (3*128)
EPS = 1e-6
CT = 22
CH = 21
CW = 21

_BF16 = ml_dtypes.bfloat16
_cache: dict = {}


def _build_theta(freqs_angle, cs):
    start_frame = cs // FRAME
    nf = S_TOTAL // FRAME
    t = freqs_angle[start_frame:start_frame + nf, :CT]
    h = freqs_angle[:H, CT:CT + CH]
    w = freqs_angle[:W, CT + CH:CT + CH + CW]
    tf = np.broadcast_to(t[:, None, None, :], (nf, H, W, CT))
    hf = np.broadcast_to(h[None, :, None, :], (nf, H, W, CH))
    wf = np.broadcast_to(w[None, None, :, :], (nf, H, W, CW))
    return np.concatenate([tf, hf, wf], axis=-1).reshape(nf * H * W, HALF)


def _build_program(n_prior, np_pad, n_pads):
    import concourse.bass as bass  # noqa: F401
    import concourse.tile as tile
    from concourse import bacc, mybir
    from concourse.masks import make_identity

    f32 = mybir.dt.float32
    f32r = mybir.dt.float32r
    bf16 = mybir.dt.bfloat16
    Act = mybir.ActivationFunctionType
    Alu = mybir.AluOpType

    NPT = np_pad // 128          # prior l-tiles (21)
    CLT = SCP // 128             # current l-tiles per source core (3)
    NK = DIM // 128              # 12 contraction chunks
    sm_scale = 1.0 / math.sqrt(HD)

    nc = bacc.Bacc("TRN2", target_bir_lowering=False, debug=False,
                   num_devices=NC)

    xT = nc.dram_tensor("xT", [DIM, SC], bf16, kind="ExternalInput").ap()
    thetaT = nc.dram_tensor("thetaT", [HALF, SC], f32, kind="ExternalInput").ap()
    wq = nc.dram_tensor("wq", [DIM, DIM], bf16, kind="ExternalInput").ap()
    wk = nc.dram_tensor("wk", [DIM, DIM], bf16, kind="ExternalInput").ap()
    wv = nc.dram_tensor("wv", [DIM, DIM], bf16, kind="ExternalInput").ap()
    wo = nc.dram_tensor("wo", [DIM, DIM], bf16, kind="ExternalInput").ap()
    bq2 = nc.dram_tensor("bq2", [HD, NH], f32, kind="ExternalInput").ap()
    bk2 = nc.dram_tensor("bk2", [HD, NH], f32, kind="ExternalInput").ap()
    gq2 = nc.dram_tensor("gq2", [HD, NH], f32, kind="ExternalInput").ap()
    gk2 = nc.dram_tensor("gk2", [HD, NH], f32, kind="ExternalInput").ap()
    bv1 = nc.dram_tensor("bv1", [1, DIM], bf16, kind="ExternalInput").ap()
    bo1 = nc.dram_tensor("bo1", [1, DIM], bf16, kind="ExternalInput").ap()
    pswT = nc.dram_tensor("pswT", [HD, HD], bf16, kind="ExternalInput").ap()
    priorKT = nc.dram_tensor("priorKT", [NH, HD, np_pad], bf16,
                             kind="ExternalInput").ap()
    priorVT = nc.dram_tensor("priorVT", [NH, np_pad, HD], bf16,
                             kind="ExternalInput").ap()
    out = nc.dram_tensor("out", [SC, DIM], f32, kind="ExternalOutput").ap()

    def r32(ap):
        return ap.bitcast(f32r)

    with tile.TileContext(nc, trace_sim=False) as tc, ExitStack() as ctx:
        consts = ctx.enter_context(tc.tile_pool(name="consts", bufs=1))
        wstr = ctx.enter_context(tc.tile_pool(name="wstr", bufs=3))
        xpool = ctx.enter_context(tc.tile_pool(name="xpool", bufs=1))
        acts = ctx.enter_context(tc.tile_pool(name="acts", bufs=1))
        sqp = ctx.enter_context(tc.tile_pool(name="sqp", bufs=2))
        csrp = ctx.enter_context(tc.tile_pool(name="csrp", bufs=2))
        kvs = ctx.enter_context(tc.tile_pool(name="kvs", bufs=2))
        escp = ctx.enter_context(tc.tile_pool(name="escp", bufs=4))
        smal = ctx.enter_context(tc.tile_pool(name="smal", bufs=4))
        outp = ctx.enter_context(tc.tile_pool(name="outp", bufs=1))
        dram = ctx.enter_context(tc.tile_pool(name="dram", bufs=1, space="DRAM"))
        pp = ctx.enter_context(tc.tile_pool(name="pp", bufs=2, space="PSUM"))
        psw = ctx.enter_context(tc.tile_pool(name="psw", bufs=2, space="PSUM"))
        pacc = ctx.enter_context(tc.tile_pool(name="pacc", bufs=3, space="PSUM"))
        pmisc = ctx.enter_context(tc.tile_pool(name="pmisc", bufs=1, space="PSUM"))

        # ---------- constants ----------
        _constv_cache = {}

        def constv(val):
            if val not in _constv_cache:
                t = consts.tile([128, 1], f32, name=f"cv_{len(_constv_cache)}")
                nc.vector.memset(t, val)
                _constv_cache[val] = t
            return _constv_cache[val]

        ident = consts.tile([128, 128], f32)
        make_identity(nc, ident)
        ones_col = consts.tile([128, 1], f32)
        nc.vector.memset(ones_col, 1.0)
        ones_row = consts.tile([1, 128], bf16)
        nc.vector.memset(ones_row, 1.0)
        ones_row_f = consts.tile([1, 128], f32)
        nc.vector.memset(ones_row_f, 1.0)
        psw_sb = consts.tile([HD, HD], bf16)
        nc.sync.dma_start(psw_sb, pswT)
        th2 = consts.tile([128, SC], f32)
        nc.sync.dma_start(th2[0:HALF, :], thetaT)
        nc.sync.dma_start(th2[HALF:128, :], thetaT)
        # CC = [cos; cos], SS = [-sin; sin]
        cc = consts.tile([128, SC], f32)
        ss = consts.tile([128, SC], f32)
        nc.scalar.activation(cc, th2, Act.Sin, bias=constv(math.pi / 2.0))
        nc.scalar.activation(ss[0:HALF, :], th2[0:HALF, :], Act.Sin, scale=constv(-1.0)[0:HALF])
        nc.scalar.activation(ss[HALF:128, :], th2[HALF:128, :], Act.Sin)
        bq_sb = consts.tile([HD, NH], f32)
        bk_sb = consts.tile([HD, NH], f32)
        gq_sb = consts.tile([HD, NH], f32)
        gk_sb = consts.tile([HD, NH], f32)
        nc.sync.dma_start(bq_sb, bq2)
        nc.sync.dma_start(bk_sb, bk2)
        nc.sync.dma_start(gq_sb, gq2)
        nc.sync.dma_start(gk_sb, gk2)
        bqg = consts.tile([HD, NH], f32)
        bkg = consts.tile([HD, NH], f32)
        nc.vector.tensor_mul(bqg, bq_sb, gq_sb)
        nc.vector.tensor_mul(bkg, bk_sb, gk_sb)
        bv_sb = consts.tile([1, DIM], bf16)
        bo_sb = consts.tile([1, DIM], bf16)
        nc.sync.dma_start(bv_sb, bv1)
        nc.sync.dma_start(bo_sb, bo1)
        zpad = consts.tile([64, HD], bf16)
        nc.vector.memset(zpad, 0.0)

        # ---------- x ----------
        xs = xpool.tile([128, NK, SC], bf16)
        nc.sync.dma_start(xs, xT.rearrange("(ko ki) t -> ki ko t", ki=128))

        # ---------- internal DRAM for collectives ----------
        k_cc_in = dram.tile([NH, HD, SCP], bf16)
        v_cc_in = dram.tile([NH, SCP, HD], bf16)
        kg = dram.tile([NC, NH, HD, SCP], bf16, addr_space="Shared")
        vg = dram.tile([NC, NH, SCP, HD], bf16, addr_space="Shared")
        rgroups = [list(range(NC))]

        w_re = "(ko ki) m -> ki ko m"

        # ---------- projection helper (q / k): [d, t] + norm factors ------
        def qk_projection(w_dram, b_sb, g_sb, bg_sb, name):
            raw = acts.tile([128, NH, SC], bf16, tag="raw", name=f"raw_{name}")
            pss = pmisc.tile([128, SC], f32, tag="pss", name=f"pss_{name}")
            for m in range(NH):
                wm = wstr.tile([128, NK, 128], bf16, tag="wm",
                               name=f"wm_{name}_{m}")
                nc.sync.dma_start(
                    wm, w_dram.rearrange(w_re, ki=128)[:, :, m * 128:(m + 1) * 128])
                ps = pp.tile([128, 512], f32, tag="pp", name=f"pj_{name}_{m}")
                for kk in range(NK):
                    nc.tensor.matmul(
                        ps[:, :SC], wm[:, kk, :], xs[:, kk, :],
                        start=(kk == 0), stop=(kk == NK - 1))
                nc.scalar.activation(raw[:, m, :], ps[:, :SC], Act.Identity,
                                     bias=bg_sb[:, m:m + 1],
                                     scale=g_sb[:, m:m + 1])
                sq = sqp.tile([128, SC], f32, tag="sq")
                nc.scalar.activation(sq, ps[:, :SC], Act.Square,
                                     bias=b_sb[:, m:m + 1])
                nc.tensor.matmul(pss[0:1, :], ones_col, sq,
                                 start=(m == 0), stop=(m == NH - 1))
            r1 = smal.tile([1, SC], f32, tag="r1")
            nc.scalar.activation(r1, pss[0:1, :], Act.Sqrt,
                                 scale=constv(1.0 / DIM)[0:1],
                                 bias=constv(EPS)[0:1])
            rr = smal.tile([1, SC], f32, tag="rr")
            nc.vector.reciprocal(rr, r1)
            rrb = psw.tile([128, 512], f32, tag="psw", name=f"rrb_{name}")
            nc.tensor.matmul(rrb[:, :SC], ones_row_f, rr,
                             start=True, stop=True)
            ccr = csrp.tile([128, SC], f32, tag="ccr")
            ssr = csrp.tile([128, SC], f32, tag="ssr")
            nc.vector.tensor_mul(ccr, cc, rrb[:, :SC])
            nc.vector.tensor_mul(ssr, ss, rrb[:, :SC])
            return raw, ccr, ssr

        def rope_chunk(raw, ccr, ssr, m, dst_ap, name):
            # dst = raw*ccr + swap_halves(raw)*ssr   (swap via PE matmul)
            pw = psw.tile([128, 512], f32, tag="psw", name=f"sw_{name}_{m}")
            nc.tensor.matmul(pw[:, :SC], psw_sb, raw[:, m, :],
                             start=True, stop=True)
            m1 = sqp.tile([128, SC], f32, tag="m1")
            nc.vector.tensor_mul(m1, raw[:, m, :], ccr)
            m2 = sqp.tile([128, SC], f32, tag="m2")
            nc.vector.tensor_mul(m2, pw[:, :SC], ssr)
            nc.vector.tensor_add(dst_ap, m1, m2)

        # ---------- K ----------
        raw_k, ccr_k, ssr_k = qk_projection(wk, bk_sb, gk_sb, bkg, "k")
        kn = acts.tile([128, NH, SCP], bf16)
        nc.vector.memset(kn, 0.0)
        for m in range(NH):
            rope_chunk(raw_k, ccr_k, ssr_k, m, kn[:, m, :SC], "k")
        for m in range(NH):
            nc.sync.dma_start(k_cc_in[m], kn[:, m, :])
        nc.gpsimd.collective_compute(
            "AllGather", Alu.bypass, replica_groups=rgroups,
            ins=[k_cc_in.opt()], outs=[kg.opt()])

        # ---------- V (direct [t, d] production) ----------
        vt = acts.tile([128, 3, DIM], bf16)
        for oc in range(3):
            pvs = [pacc.tile([128, 512], f32, tag="pacc",
                             name=f"pv_{oc}_{tci}") for tci in range(3)]
            for kk in range(NK):
                wc = wstr.tile([128, 512], bf16, tag="wc", name=f"wv_{oc}_{kk}")
                nc.sync.dma_start(
                    wc, wv.rearrange(w_re, ki=128)[:, kk, oc * 512:(oc + 1) * 512])
                for tci in range(3):
                    nc.tensor.matmul(
                        pvs[tci][:ST, :],
                        xs[:, kk, tci * ST:(tci + 1) * ST], wc,
                        start=(kk == 0), stop=False)
            for tci in range(3):
                nc.tensor.matmul(
                    pvs[tci][:ST, :], ones_row[:, :ST],
                    bv_sb[:, oc * 512:(oc + 1) * 512],
                    start=False, stop=True)
                nc.vector.tensor_copy(
                    vt[:ST, tci, oc * 512:(oc + 1) * 512], pvs[tci][:ST, :])
        for h in range(NH):
            nc.sync.dma_start(
                v_cc_in[h, 0:SC, :].rearrange("(tc p) d -> p tc d", p=ST),
                vt[:ST, :, h * 128:(h + 1) * 128])
            nc.sync.dma_start(v_cc_in[h, SC:SCP, :], zpad[0:SCP - SC, :])
        nc.gpsimd.collective_compute(
            "AllGather", Alu.bypass, replica_groups=rgroups,
            ins=[v_cc_in.opt()], outs=[vg.opt()])

        # ---------- Q ----------
        raw_q, ccr_q, ssr_q = qk_projection(wq, bq_sb, gq_sb, bqg, "q")
        qn = acts.tile([128, NH, SC], bf16)
        for m in range(NH):
            rope_chunk(raw_q, ccr_q, ssr_q, m, qn[:, m, :], "q")

        # ---------- attention ----------
        part1 = outp.tile([128, NH, 3, 130], f32)
        oT = outp.tile([128, NH, SC], bf16)

        def attn_accum(h, lhsT_tiles, v_tiles, n_tiles, phase):
            pos = [pacc.tile([128, 129], f32, tag="pacc",
                             name=f"po_{phase}_{h}_{si}") for si in range(3)]
            for lt in range(n_tiles):
                ps = pp.tile([128, 512], f32, tag="pp",
                             name=f"sc_{phase}_{h}_{lt}")
                nc.tensor.matmul(ps[:, :SC], lhsT_tiles(lt), qn[:, h, :],
                                 start=True, stop=True)
                esc = escp.tile([128, SC], bf16, tag="esc")
                nc.scalar.activation(esc, ps[:, :SC], Act.Exp, scale=constv(sm_scale))
                for si in range(3):
                    nc.tensor.matmul(
                        pos[si][:ST, :],
                        esc[:, si * ST:(si + 1) * ST],
                        v_tiles(lt),
                        start=(lt == 0), stop=(lt == n_tiles - 1))
            for si in range(3):
                if phase == "p":
                    nc.vector.tensor_copy(part1[:ST, h, si, :129],
                                          pos[si][:ST, :])
                else:
                    nc.vector.tensor_add(part1[:ST, h, si, :129],
                                         pos[si][:ST, :],
                                         part1[:ST, h, si, :129])

        # pass 1: prior KV (overlaps the AllGathers)
        for h in range(NH):
            pkh = kvs.tile([128, np_pad], bf16, tag="kload")
            nc.sync.dma_start(pkh, priorKT[h])
            pvh = kvs.tile([128, NPT, 130], bf16, tag="vload")
            nc.sync.dma_start(
                pvh[:, :, 0:HD],
                priorVT[h].rearrange("(lt p) d -> p lt d", p=128))
            nc.vector.memset(pvh[:, :, 128:129], 1.0)
            attn_accum(
                h,
                lambda lt, pkh=pkh: pkh[:, lt * 128:(lt + 1) * 128],
                lambda lt, pvh=pvh: pvh[:, lt, 0:129],
                NPT, "p")

        # pass 2: current KV (needs AllGather results)
        for h in range(NH):
            kgh = kvs.tile([128, NC, SCP], bf16, tag="kload")
            nc.sync.dma_start(kgh, kg[:, h].rearrange("c p t -> p c t"))
            vgh = kvs.tile([128, NC, CLT, 130], bf16, tag="vload")
            for cb in range(NC):
                nc.sync.dma_start(
                    vgh[:, cb, :, 0:HD],
                    vg[cb, h].rearrange("(lt p) d -> p lt d", p=128))
            nc.vector.memset(vgh[:, :, :, 128:129], 1.0)
            attn_accum(
                h,
                lambda lt, kgh=kgh: kgh[:, lt // CLT,
                                        (lt % CLT) * 128:(lt % CLT + 1) * 128],
                lambda lt, vgh=vgh: vgh[:, lt // CLT, lt % CLT, 0:129],
                NC * CLT, "c")
            # divide by corrected denominator; transpose to [d, t]
            for si in range(3):
                den = smal.tile([128, 1], f32, tag="den")
                nc.vector.tensor_scalar_add(den[:ST, :],
                                            part1[:ST, h, si, 128:129],
                                            -float(n_pads))
                rcp = smal.tile([128, 1], f32, tag="rcp")
                nc.vector.reciprocal(rcp[:ST, :], den[:ST, :])
                odiv = sqp.tile([128, 128], f32, tag="odiv")
                nc.scalar.activation(odiv[:ST, :], part1[:ST, h, si, 0:128],
                                     Act.Copy, scale=rcp[:ST, 0:1])
                ptr = psw.tile([128, 512], f32, tag="psw",
                               name=f"ptr_{h}_{si}")
                nc.tensor.transpose(ptr[:, :ST], odiv[:ST, :],
                                    ident[:ST, :ST])
                nc.vector.tensor_copy(oT[:, h, si * ST:(si + 1) * ST],
                                      ptr[:, :ST])

        # ---------- output projection ----------
        for oc in range(3):
            pos = [pacc.tile([128, 512], f32, tag="pacc",
                             name=f"pout_{oc}_{tci}") for tci in range(3)]
            for h in range(NH):
                wc = wstr.tile([128, 512], bf16, tag="wc", name=f"wo_{oc}_{h}")
                nc.sync.dma_start(
                    wc, wo.rearrange(w_re, ki=128)[:, h, oc * 512:(oc + 1) * 512])
                for tci in range(3):
                    nc.tensor.matmul(
                        pos[tci][:ST, :],
                        oT[:, h, tci * ST:(tci + 1) * ST], wc,
                        start=(h == 0), stop=False)
            for tci in range(3):
                nc.tensor.matmul(
                    pos[tci][:ST, :], ones_row[:, :ST],
                    bo_sb[:, oc * 512:(oc + 1) * 512],
                    start=False, stop=True)
                ob = sqp.tile([128, 512], f32, tag="ob")
                nc.vector.tensor_copy(ob[:ST, :], pos[tci][:ST, :])
                nc.sync.dma_start(
                    out[tci * ST:(tci + 1) * ST, oc * 512:(oc + 1) * 512],
                    ob[:ST, :])

    nc.compile()
    return nc


def _prep(inputs):
    x = np.asarray(inputs["x"], np.float32)
    freqs_angle = np.asarray(inputs["freqs_angle"], np.float32)
    prior_k = np.asarray(inputs["prior_k"], np.float32)
    prior_v = np.asarray(inputs["prior_v"], np.float32)
    cs = int(np.asarray(inputs["current_start"]))

    block = 3 * FRAME
    block_end = (cs // block + 1) * block
    keep_from = max(0, block_end - 6 * FRAME)
    keep_size = min(cs + S_TOTAL - keep_from, prior_k.shape[0] + S_TOTAL)
    n_prior = keep_size - S_TOTAL
    p0 = prior_k.shape[0] - n_prior
    np_pad = -(-n_prior // 128) * 128
    n_pads = (np_pad - n_prior) + NC * (SCP - SC)

    perm = np.concatenate(
        [h * HD + np.concatenate([np.arange(0, HD, 2), np.arange(1, HD, 2)])
         for h in range(NH)])

    WqT = np.ascontiguousarray(np.asarray(inputs["Wq"], np.float32)[perm].T).astype(_BF16)
    WkT = np.ascontiguousarray(np.asarray(inputs["Wk"], np.float32)[perm].T).astype(_BF16)
    WvT = np.ascontiguousarray(np.asarray(inputs["Wv"], np.float32).T).astype(_BF16)
    WoT = np.ascontiguousarray(np.asarray(inputs["Wo"], np.float32).T).astype(_BF16)

    def two(vec, p=None):
        v = np.asarray(vec, np.float32)
        if p is not None:
            v = v[p]
        return np.ascontiguousarray(v.reshape(NH, HD).T)

    bq2 = two(inputs["bq"], perm)
    bk2 = two(inputs["bk"], perm)
    gq2 = two(inputs["gq"], perm)
    gk2 = two(inputs["gk"], perm)
    bv1 = np.asarray(inputs["bv"], np.float32).reshape(1, DIM).astype(_BF16)
    bo1 = np.asarray(inputs["bo"], np.float32).reshape(1, DIM).astype(_BF16)

    pswT = np.zeros((HD, HD), _BF16)
    for r in range(HD):
        pswT[(r + HALF) % HD, r] = 1.0   # lhsT of the half-swap permutation

    theta = _build_theta(freqs_angle, cs)          # [S, 64]
    thetaT = np.ascontiguousarray(theta.T)

    pk = prior_k[p0:].reshape(n_prior, DIM)[:, perm]
    priorKT = np.zeros((DIM, np_pad), np.float32)
    priorKT[:, :n_prior] = pk.T
    priorKT = np.ascontiguousarray(priorKT.reshape(NH, HD, np_pad)).astype(_BF16)
    priorVT = np.zeros((NH, np_pad, HD), np.float32)
    priorVT[:, :n_prior, :] = prior_v[p0:].transpose(1, 0, 2)
    priorVT = priorVT.astype(_BF16)

    xT = np.ascontiguousarray(x[0].T).astype(_BF16)              # [DIM, S]

    shared = dict(wq=WqT, wk=WkT, wv=WvT, wo=WoT, bq2=bq2, bk2=bk2,
                  gq2=gq2, gk2=gk2, bv1=bv1, bo1=bo1, pswT=pswT,
                  priorKT=priorKT, priorVT=priorVT)
    in_maps = []
    for c in range(NC):
        m = dict(shared)
        m["xT"] = np.ascontiguousarray(xT[:, c * SC:(c + 1) * SC])
        m["thetaT"] = np.ascontiguousarray(thetaT[:, c * SC:(c + 1) * SC])
        in_maps.append(m)
    return in_maps, (n_prior, np_pad, n_pads)


def kernel(**inputs) -> np.ndarray:
    import os
    from concourse.bass_utils import run_bass_kernel_spmd

    in_maps, key = _prep(inputs)
    if key not in _cache:
        _cache[key] = _build_program(*key)
    nc = _cache[key]

    trace = bool(int(os.environ.get("KERNEL_TRACE", "0")))
    try:
        res = run_bass_kernel_spmd(
            nc, in_maps, core_ids=list(range(NC)), trace=trace,
            trace_cores=list(range(NC)) if trace else None)
    except ModuleNotFoundError:
        res = run_bass_kernel_spmd(nc, in_maps, core_ids=list(range(NC)),
                                   trace=False)
    kernel.last_results = res
    outp = np.concatenate([res.results[c]["out"] for c in range(NC)], axis=0)
    return outp[None].astype(np.float32)

